# revision 1
# baseline (speedup 1.0000x reference)
"""KANLayer kernel for 8 Trainium2 NeuronCores (raw Bass, explicit semaphores).

Reference computation (B=4096, D=1024, O=1024, S=4 spline points):
    xmin/xmax = per-feature min/max of x over the batch dim      # [1, D]
    xn  = (x - xmin) / (xmax - xmin)                             # [B, D]
    c   = spline_coeffs.sum(axis=2)                              # [O, D, 4]
    out = xn^3 @ c0.T + xn^2 @ c1.T + xn @ c2.T + c3.sum(d)     # [B, O]

Sharding: tensor-parallel over the output dim O. Core r owns output columns
[128r, 128r+128). x is needed in full by every core (the contraction runs
over all D features); the host transposes it once to xT [D, B] so loads land
d-major (the matmul contracts over the SBUF partition dim = d).

The batch min/max is sharded: core r reduces only its own 128-feature slice
(fed separately as "xmine"), converts to scale/bias (s = 1/(max-min),
beta = -min*s), and an 8-core AllGather (1 KB payload) distributes all
[D, 2] scale/bias pairs to every core.

Per d-chunk j (128 features), full batch width:
    SP  : DMA xT[128j:128j+128, :]            -> xt slot      [128, 4096]
    ACT : xn = Relu(xt * s_j + beta_j)        (xn >= 0, so Relu == copy)
    ACT : x2 = Square(xn)
    DVE : x3 = xn * x2
    PE  : psum[:, 512q:...] += c_k[j].T @ pow_k   k in {0,1,2}, q in 0..7
Spline-coefficient prep runs on the DMA engines: the host supplies the shard
as [S, 4, D, 128]; two parallel 2-deep SWDGE accumulate chains (copy +
accum_op=add) pair-sum the spline planes, and one DVE add merges the pairs
while rounding fp32 -> float32r (walrus requires f32r matmul operands be
*written* as f32r). The constant term sum_d c3[o,d] comes from a
natural-layout copy of the k=3 plane ("c3nat") reduced on DVE, added during
the PSUM->SBUF drains.

Matmuls run in float32r (fp32 bits, FP22 truncation inside the PE): 1 PE
cycle/row at N=512 (4x faster than true fp32) with ~2^-14 input rounding
(measured end-to-end relative error ~1.4e-4).

Toolchain constraints honored here:
  * this walrus lowers at most ONE semaphore wait per instruction, so the
    Tile scheduler (multi-wait sync_info) is unusable; every wait below is a
    standalone wait_ge instruction;
  * the sim race detector does not credit same-engine program order, so
    intra-engine data deps carry explicit self-sem chains;
  * a DMA's then_inc(sem, 16) lands as 16 separate +1s, so concurrently
    in-flight DMAs must use different semaphores (s_xte/s_xto parity split).

n_iters > 1 builds a NEFF that runs the whole kernel N times back-to-back
(for device-time measurement by wall-clock slope; the axon tunnel's per-call
input shipping makes single-run wall time meaningless).

Output per core is out_t [128, B] (transposed); the host concatenates the
8 shards and transposes back.
"""

import numpy as np

import concourse.bass as bass
import concourse.mybir as mybir
from concourse.bass_utils import run_bass_kernel_spmd

P = 128            # SBUF partitions / rows per tile
B = 4096           # batch
D = 1024           # input features
O = 1024           # output features
S = 4              # spline points
KC = 4             # cubic coefficients per (o, d)
NCORES = 8
OS = O // NCORES   # output columns per core = 128
DC = D // P        # d-chunks = 8
QW = 512           # matmul moving-dim width (one PSUM bank)
NQ = B // QW       # 8
X2W = 2560         # xn^2 columns squared on ACT; the rest on DVE (balance)

F32 = mybir.dt.float32
F32R = mybir.dt.float32r
AX = mybir.AxisListType
ALU = mybir.AluOpType
ACTF = mybir.ActivationFunctionType

_CACHE = {}


def _build_bass(n_iters: int = 1, timing_mode: bool = False) -> bass.Bass:
    nc = bass.Bass(num_devices=NCORES)

    # timing_mode: all big tensors internal (uninitialized, contents don't
    # affect timing) so the axon tunnel ships ~nothing per call; one tiny
    # external output keeps the PJRT contract. Used only to measure device
    # time by NEFF-repeat slope.
    kind = {} if timing_mode else {"kind": "ExternalInput"}
    okind = {} if timing_mode else {"kind": "ExternalOutput"}
    xt = nc.dram_tensor("xt", [D, B], F32, **kind)
    xmine = nc.dram_tensor("xmine", [P, B], F32, **kind)
    # [S, KC, D, OS]: s-major so each spline plane is one contiguous DMA
    coeffs = nc.dram_tensor("coeffs", [S, KC, D, OS], F32, **kind)
    # natural-layout k=3 plane for the bias reduction: [o, d*s]
    c3nat = nc.dram_tensor("c3nat", [OS, D * S], F32, **kind)
    out_t = nc.dram_tensor("out_t", [OS, B], F32, **okind)
    dummy = (
        nc.dram_tensor("tout", [P, 2], F32, kind="ExternalOutput")
        if timing_mode
        else None
    )
    cc_in = nc.dram_tensor("cc_in", [P, 2], F32)
    cc_all = nc.dram_tensor("cc_all", [D, 2], F32, addr_space="Shared")

    from contextlib import ExitStack

    ctx = ExitStack()
    with ctx:
        sem = lambda name: ctx.enter_context(nc.semaphore(name))  # noqa: E731
        s_xm = sem("s_xm")        # +16/iter: xmine load
        s_xte = sem("s_xte")      # +16 per even-chunk xt load (slot 0)
        s_xto = sem("s_xto")      # +16 per odd-chunk xt load (slot 1)
        s_cfa = sem("s_cfa")      # +16 per coeff plane DMA (pair A: s0, s1)
        s_cfb = sem("s_cfb")      # +16 per coeff plane DMA (pair B: s2, s3)
        s_cc = sem("s_cc")        # +1/iter: collective done
        s_sball = sem("s_sball")  # +16/iter: sball load
        s_ccin = sem("s_ccin")    # +16/iter: cc_in store
        s_c3n = sem("s_c3n")      # +16/iter: c3nat load
        s_stats = sem("s_stats")  # +1/iter: scale/bias computed
        s_act = sem("s_act")      # +1 per ACT op (xn, x2): 16/iter
        s_x3 = sem("s_x3")        # +1 per chunk: 8/iter
        s_pe = sem("s_pe")        # +1 per finished matmul chunk: 8/iter
        s_conv = sem("s_conv")    # +1/iter: c_allr ready
        s_drain = sem("s_drain")  # +1 per psum->sbuf drain: 8/iter
        s_out = sem("s_out")      # +16 per output DMA: 128/iter
        s_dv = sem("s_dv")        # DVE same-engine retirement chain: 6/iter
        s_dx = sem("s_dx")        # +1 per DVE x2-half: 8/iter
        s_fin = sem("s_fin")      # timing_mode dummy-output store

        sb = lambda name, shape, dtype=F32: ctx.enter_context(  # noqa: E731
            nc.sbuf_tensor(name, shape, dtype)
        )
        xt_sb = [sb(f"xt{i}", [P, B]) for i in range(2)]
        xn_sb = [sb(f"xn{i}", [P, B], F32R) for i in range(2)]
        x2_sb = [sb(f"x2{i}", [P, B], F32R) for i in range(2)]
        x3_sb = [sb(f"x3{i}", [P, B], F32R) for i in range(2)]
        cpa = sb("cpa", [P, KC, DC, OS])
        c_allr = sb("c_allr", [P, KC, DC, OS], F32R)
        c3n_sb = sb("c3n", [P, D * S])
        sball = sb("sball", [P, DC, 2])
        sb2 = sb("sb2", [P, 2])
        mn = sb("mn", [P, 1])
        mx = sb("mx", [P, 1])
        rng = sb("rng", [P, 1])
        bias_sb = sb("bias_sb", [P, 1])
        # output staging; its buffer doubles as the second spline-pair
        # accumulator (cpb): cpb's last read (the DVE pair-add) precedes the
        # first drain write in DVE program order.
        out_all = sb("out_all", [P, B])
        out_sb = [out_all[:, q * QW : (q + 1) * QW] for q in range(NQ)]

        psum = ctx.enter_context(nc.psum_tensor("ps", [P, B], F32))

        cpb = out_all[:, :].rearrange("p (k j o) -> p k j o", k=KC, j=DC)
        c_all_r = c_allr[:, :, :, :]

        NI = n_iters
        CC1 = 16 if timing_mode else 1  # s_cc increment per iteration

        with nc.Block() as block:

            @block.sync
            def _(sp):
                if timing_mode:
                    # overwrite garbage DRAM with finite data once (slope-
                    # neutral: identical intercept in the 1x and Nx builds)
                    sp.wait_ge(s_fin, 1)  # out_all memset (the 0.3 source)
                    zsrc = out_all[:, :]
                    sp.dma_start(
                        out=xt[:, :].rearrange("(n p) f -> p n f", p=P),
                        in_=bass.AP(
                            tensor=zsrc.tensor,
                            offset=zsrc.offset,
                            ap=[[zsrc.ap[0][0], P], [0, D // P], [1, B]],
                        ),
                    ).then_inc(s_fin, 16)
                    sp.wait_ge(s_fin, 17)
                    sp.dma_start(out=xmine[:, :], in_=zsrc).then_inc(s_fin, 16)
                    sp.wait_ge(s_fin, 33)
                    sp.dma_start(out=c3nat[:, :], in_=zsrc).then_inc(s_fin, 16)
                    sp.wait_ge(s_fin, 49)
                    nflat = S * KC * D * OS // P  # 16384 per partition
                    sp.dma_start(
                        out=coeffs[:, :, :, :]
                        .rearrange("s k d o -> (s k d o)")
                        .rearrange("(p f) -> p f", p=P)
                        .rearrange("p (m f) -> p m f", f=B),
                        in_=bass.AP(
                            tensor=zsrc.tensor,
                            offset=zsrc.offset,
                            ap=[[zsrc.ap[0][0], P], [0, nflat // B], [1, B]],
                        ),
                    ).then_inc(s_fin, 16)
                for it in range(NI):
                    A = 16 * it       # s_act ticks before this iteration
                    if it > 0:
                        sp.wait_ge(s_act, A)       # xt slots + sball free
                        sp.wait_ge(s_cc, CC1 * it)
                        sp.wait_ge(s_dv, (3 if timing_mode else 6) * it)
                    sp.dma_start(out=xt_sb[1][:, :], in_=xmine[:, :]).then_inc(
                        s_xm, 16
                    )
                    sp.dma_start(out=xt_sb[0][:, :], in_=xt[0:P, :]).then_inc(
                        s_xte, 16
                    )
                    sp.dma_start(out=c3n_sb[:, :], in_=c3nat[:, :]).then_inc(
                        s_c3n, 16
                    )
                    # chunk 1 overwrites the xmine data: wait for the stats
                    sp.wait_ge(s_stats, it + 1)
                    sp.dma_start(out=cc_in[:, :], in_=sb2[:, :]).then_inc(
                        s_ccin, 16
                    )
                    sp.dma_start(
                        out=xt_sb[1][:, :], in_=xt[P : 2 * P, :]
                    ).then_inc(s_xto, 16)
                    sp.wait_ge(s_cc, CC1 * (it + 1))
                    sp.dma_start(
                        out=sball[:, :, :],
                        in_=cc_all[:, :].rearrange("(j p) c -> p j c", p=P),
                    ).then_inc(s_sball, 16)
                    for j in range(2, DC):
                        # slot recycle: xn(j-2) (ACT op A+2*(j-2)+1) read it
                        sp.wait_ge(s_act, A + 2 * (j - 2) + 1)
                        sp.dma_start(
                            out=xt_sb[j % 2][:, :], in_=xt[j * P : (j + 1) * P, :]
                        ).then_inc(s_xte if j % 2 == 0 else s_xto, 16)
                    for q in range(NQ):
                        sp.wait_ge(s_drain, 8 * it + q + 1)
                        sp.dma_start(
                            out=out_t[:, q * QW : (q + 1) * QW], in_=out_sb[q]
                        ).then_inc(s_out, 16)
                sp.wait_ge(s_out, 16 * NQ * NI)
                if dummy is not None:
                    sp.dma_start(out=dummy[:, :], in_=sb2[:, :]).then_inc(
                        s_fin, 16
                    )
                    sp.wait_ge(s_fin, 81)

            @block.scalar
            def _(act):
                for it in range(NI):
                    act.wait_ge(s_sball, 16 * (it + 1))
                    for j in range(DC):
                        act.wait_ge(
                            s_xte if j % 2 == 0 else s_xto,
                            16 * (4 * it + j // 2 + 1),
                        )
                        g = 8 * it + j  # global chunk index
                        if g >= 2:
                            # pow slot recycle: PE finished chunk g-2, and
                            # own x2(g-2) retired (WAR on the xn slot)
                            act.wait_ge(s_pe, g - 1)
                            act.wait_ge(s_act, 2 * g - 2)
                        act.activation(
                            xn_sb[j % 2][:, :],
                            xt_sb[j % 2][:, :],
                            ACTF.Relu,
                            bias=sball[:, j, 1:2],
                            scale=sball[:, j, 0:1],
                        ).then_inc(s_act)
                        act.wait_ge(s_act, 2 * g + 1)  # xn retired
                        act.activation(
                            x2_sb[j % 2][:, :X2W],
                            xn_sb[j % 2][:, :X2W],
                            ACTF.Square,
                        ).then_inc(s_act)

            @block.vector
            def _(dve):
                # The sim race detector does not credit same-engine program
                # order, so intra-DVE data deps carry a self-sem chain (s_dv);
                # on HW the wait_ge just confirms the prior op retired.
                DVT = 3 if timing_mode else 6
                if timing_mode:
                    dve.memset(out_all[:, :], 0.3).then_inc(s_fin)
                for it in range(NI):
                    V = DVT * it
                    dve.wait_ge(s_xm, 16 * (it + 1))
                    dve.tensor_reduce(
                        mn[:, :], xt_sb[1][:, :], axis=AX.X, op=ALU.min
                    ).then_inc(s_dv)
                    dve.tensor_reduce(
                        mx[:, :], xt_sb[1][:, :], axis=AX.X, op=ALU.max
                    ).then_inc(s_dv)
                    dve.wait_ge(s_dv, V + 2)
                    if timing_mode:
                        # uninitialized DRAM would make 1/(max-min) inf and
                        # flood the runtime's NaN notification path; force
                        # finite scale/bias instead of the tiny stats tail
                        dve.memset(sb2[:, :], 0.25).then_inc(s_stats)
                    else:
                        dve.tensor_sub(rng[:, :], mx[:, :], mn[:, :]).then_inc(
                            s_dv
                        )
                        dve.wait_ge(s_dv, V + 3)
                        dve.reciprocal(sb2[:, 0:1], rng[:, :]).then_inc(s_dv)
                        dve.wait_ge(s_dv, V + 4)
                        # mx doubles as -beta staging scratch (dead after rng)
                        dve.tensor_mul(mx[:, :], mn[:, :], sb2[:, 0:1]).then_inc(
                            s_dv
                        )
                        dve.wait_ge(s_dv, V + 5)
                        dve.tensor_scalar_mul(
                            sb2[:, 1:2], mx[:, :], -1.0
                        ).then_inc(s_stats)
                    dve.wait_ge(s_c3n, 16 * (it + 1))
                    dve.tensor_reduce(
                        bias_sb[:, :], c3n_sb[:, :], axis=AX.X, op=ALU.add
                    ).then_inc(s_dv)
                    if timing_mode:
                        dve.wait_ge(s_dv, V + 3)
                        dve.memset(bias_sb[:, :], 0.5)
                    # final spline-pair add, rounding to f32r into c_allr
                    dve.wait_ge(s_cfa, 32 * (it + 1))
                    dve.wait_ge(s_cfb, 32 * (it + 1))
                    if it > 0:
                        dve.wait_ge(s_pe, 8 * it)  # c_allr read by prev iter
                    dve.tensor_add(c_all_r, cpa[:, :, :, :], cpb).then_inc(
                        s_conv
                    )
                    for j in range(DC):
                        g = 8 * it + j
                        dve.wait_ge(s_act, 2 * g + 1)   # xn(g) ready
                        if g >= 2:
                            dve.wait_ge(s_pe, g - 1)    # slot recycle
                        # DVE squares the tail columns in parallel with ACT
                        dve.tensor_mul(
                            x2_sb[j % 2][:, X2W:],
                            xn_sb[j % 2][:, X2W:],
                            xn_sb[j % 2][:, X2W:],
                        ).then_inc(s_dx)
                        dve.wait_ge(s_act, 2 * g + 2)   # ACT x2 half ready
                        dve.wait_ge(s_dx, g + 1)        # own x2 half retired
                        dve.tensor_mul(
                            x3_sb[j % 2][:, :],
                            xn_sb[j % 2][:, :],
                            x2_sb[j % 2][:, :],
                        ).then_inc(s_x3)
                    dve.wait_ge(s_pe, 8 * (it + 1))
                    dve.wait_ge(s_dv, V + DVT)   # bias_sb retired
                    dve.wait_ge(s_conv, it + 1)  # cpb read retired (WAR)
                    if it > 0:
                        dve.wait_ge(s_out, 128 * it)  # out_all reads retired
                    for q in range(NQ):
                        dve.tensor_scalar_add(
                            out_sb[q],
                            psum[:, q * QW : (q + 1) * QW],
                            bias_sb[:, 0:1],
                        ).then_inc(s_drain)

            @block.tensor
            def _(pe):
                for it in range(NI):
                    pe.wait_ge(s_conv, it + 1)
                    if it > 0:
                        pe.wait_ge(s_drain, 8 * it)  # psum read by prev iter
                    for j in range(DC):
                        g = 8 * it + j
                        pe.wait_ge(s_x3, g + 1)
                        for k in range(3):  # 0: c0*x3, 1: c1*x2, 2: c2*xn
                            src = [x3_sb, x2_sb, xn_sb][k][j % 2]
                            for q in range(NQ):
                                mm = pe.matmul(
                                    psum[:, q * QW : (q + 1) * QW],
                                    lhsT=c_all_r[:, k, j, :],
                                    rhs=src[:, q * QW : (q + 1) * QW],
                                    start=(j == 0 and k == 0),
                                    stop=(j == DC - 1 and k == 2),
                                )
                        mm.then_inc(s_pe)

            @block.gpsimd
            def _(pool):
                if timing_mode:
                    pool.wait_ge(s_fin, 65)  # coeffs DRAM initialized
                for it in range(NI):
                    if it > 0:
                        # cpa re-written below; cpb shares out_all with the
                        # previous iteration's output staging
                        pool.wait_ge(s_conv, it)
                        pool.wait_ge(s_out, 128 * it)
                    # coefficient spline-sum: two parallel 2-deep accum chains
                    pool.dma_start(
                        out=cpa[:, :, :, :],
                        in_=coeffs[0].rearrange("k (j p) o -> p k j o", p=P),
                    ).then_inc(s_cfa, 16)
                    pool.dma_start(
                        out=cpb,
                        in_=coeffs[2].rearrange("k (j p) o -> p k j o", p=P),
                    ).then_inc(s_cfb, 16)
                    pool.wait_ge(s_ccin, 16 * (it + 1))
                    if timing_mode:
                        # collective-cost probe: stand in a local DRAM copy
                        pool.dma_start(
                            out=cc_all[0:P, :], in_=cc_in[:, :]
                        ).then_inc(s_cc, 16)
                    else:
                        pool.collective_compute(
                            "AllGather",
                            ALU.bypass,
                            replica_groups=[list(range(NCORES))],
                            ins=[cc_in[:, :]],
                            outs=[cc_all[:, :]],
                        ).then_inc(s_cc)
                    pool.wait_ge(s_cfa, 32 * it + 16)
                    pool.dma_start(
                        out=cpa[:, :, :, :],
                        in_=coeffs[1].rearrange("k (j p) o -> p k j o", p=P),
                        accum_op=ALU.add,
                    ).then_inc(s_cfa, 16)
                    pool.wait_ge(s_cfb, 32 * it + 16)
                    pool.dma_start(
                        out=cpb,
                        in_=coeffs[3].rearrange("k (j p) o -> p k j o", p=P),
                        accum_op=ALU.add,
                    ).then_inc(s_cfb, 16)

    return nc


def get_bass(n_iters: int = 1, timing_mode: bool = False) -> bass.Bass:
    key = f"nc{n_iters}_{timing_mode}"
    if key not in _CACHE:
        _CACHE[key] = _build_bass(n_iters, timing_mode)
    return _CACHE[key]


def make_in_maps(x: np.ndarray, spline_coeffs: np.ndarray):
    """Host-side sharding/marshaling only (slicing + transposes, no math)."""
    x = np.ascontiguousarray(np.asarray(x, dtype=np.float32))
    spline_coeffs = np.ascontiguousarray(np.asarray(spline_coeffs, dtype=np.float32))
    xt = np.ascontiguousarray(x.T)  # [D, B]
    in_maps = []
    for r in range(NCORES):
        shard = spline_coeffs[r * OS : (r + 1) * OS]  # [OS, D, S, KC]
        in_maps.append(
            {
                "xt": xt,
                "xmine": np.ascontiguousarray(xt[r * P : (r + 1) * P]),
                # [S, KC, D, OS]
                "coeffs": np.ascontiguousarray(shard.transpose(2, 3, 1, 0)),
                # [OS, D*S] (k=3 plane, natural layout)
                "c3nat": np.ascontiguousarray(shard[:, :, :, 3]).reshape(OS, D * S),
            }
        )
    return in_maps


def assemble_output(results) -> np.ndarray:
    out = np.concatenate([results[r]["out_t"] for r in range(NCORES)], axis=0)
    return np.ascontiguousarray(out.T)  # [B, O]


def run(x: np.ndarray, spline_coeffs: np.ndarray, trace: bool = False,
        n_iters: int = 1):
    """Returns (output, BassKernelResults)."""
    nc = get_bass(n_iters)
    in_maps = make_in_maps(x, spline_coeffs)
    res = run_bass_kernel_spmd(nc, in_maps, list(range(NCORES)), trace=trace)
    return assemble_output(res.results), res


def kernel(x: np.ndarray, spline_coeffs: np.ndarray) -> np.ndarray:
    out, _ = run(x, spline_coeffs, trace=False)
    return out

